# revision 1
# baseline (speedup 1.0000x reference)
"""DeepseekV3.2 sparse MLA attention — Trainium2 Bass kernel, 8-core SPMD.

Sharding: queries row-sharded (256 rows/core); keys/values replicated.
Indexer k_idx/w_idx computed fp32 token-sharded + AllGather; indexer scores
via f16 hi/lo split matmuls (exact top-k mask); attention path f16.
Top-k realized as a per-row threshold found by bisection counting on the DVE.
"""
import numpy as np

import concourse.bass as bass
import concourse.bacc as bacc
import concourse.mybir as mybir
import concourse.tile as tile
import concourse.dve_ops as dve_ops_mod
from concourse.bass_utils import run_bass_kernel_spmd
from concourse.dve_spec import Spec, Src0, Src1, C0, relu, select, lower, spec_leaves
from concourse.dve_ops import DveOp, OPS, get_dve_sub_opcode
from concourse.dve_uop import DveOpSpec

dt = mybir.dt
AF = mybir.ActivationFunctionType
OP = mybir.AluOpType
AX = mybir.AxisListType

T, HID, H, NOPE, ROPE, VD = 2048, 7168, 16, 128, 64, 128
QLR, KVLR, IH, ID, TOPK = 1536, 512, 32, 128, 512
EPS = 1e-6
SCALE = float((NOPE + ROPE) ** -0.5)
ROPE_BASE = 10000.0
NC = 8
R = T // NC            # 256 local rows
NEG = -3.0e38
SEARCH_ITERS = 20
KT_H, KT_Q, KT_KV = HID // 128, QLR // 128, KVLR // 128


def _register(op):
    for o in OPS:
        if o.name == op.name:
            return o
    OPS.append(op)
    dve_ops_mod.CUSTOM_DVE_SPECS[op.name] = op.spec
    dve_ops_mod._SUB_OPCODE_FOR_NAME[op.name] = (
        dve_ops_mod._CUSTOM_DVE_ROW_BASE + len(OPS) - 1)
    for ver in ("v3", "v4"):
        sp = DveOpSpec(name=op.name, opcode=get_dve_sub_opcode(op.name),
                       uops=lower(op.spec, ver=ver),
                       rd1_en=Src1 in spec_leaves(op.spec))
        op.uops_sha[ver] = sp.sha(ver)
    return op


ACC_W_RELU = _register(DveOp(
    "ACC_W_RELU",
    Spec(body=Src1 + C0 * relu(Src0),
         reference=lambda in0, in1, s0, s1, imm2: in1 + s0 * np.maximum(in0, 0)),
    subdim=False, uops_sha={}))

MASKSEL = _register(DveOp(
    "MASKSEL",
    Spec(body=select(Src1, Src0, C0),
         reference=lambda in0, in1, s0, s1, imm2: np.where(in1 != 0, in0, s0)),
    subdim=False, uops_sha={}))


def build_nc():
    nc = bacc.Bacc(None)
    f32, f16 = dt.float32, dt.float16

    def din(name, shape, d=f32):
        return nc.dram_tensor(name, list(shape), d, kind="ExternalInput")

    qcT = din("qcT", [QLR, R])
    kvcT = din("kvcT", [KVLR, T])
    hT = din("hT", [HID, R])
    wikww = din("wikww", [HID, ID + IH])
    wiq_hi = din("wiq_hi", [QLR, IH * ID], f16)
    wiq_lo = din("wiq_lo", [QLR, IH * ID], f16)
    wq = din("wq", [QLR, H * (NOPE + ROPE)], f16)
    wkv_k = din("wkv_k", [KVLR, H * NOPE], f16)
    wkv_v = din("wkv_v", [KVLR, H * VD], f16)
    wo = din("wo", [H * VD, HID], f16)
    kperT = din("kperT", [ROPE, T], f16)
    cosT = din("cosT", [128, R])
    sinT = din("sinT", [128, R])
    cosK = din("cosK", [R, 32])
    sinK = din("sinK", [R, 32])
    gam_d = din("gam_rep", [128, ID])
    bet_d = din("bet_rep", [128, ID])
    idf32 = din("idf32", [128, 128])
    idf16 = din("idf16", [128, 128], f16)
    caus_d = din("causal01", [2, 128, T], f16)
    out_d = nc.dram_tensor("out", [R, HID], f32, kind="ExternalOutput")

    with tile.TileContext(nc) as tc:
        cst = tc.alloc_tile_pool(name="cst", bufs=1)
        ones16 = cst.tile([128, 128], f16); nc.vector.memset(ones16[:], 1.0)
        ones32 = cst.tile([128, 1], f32); nc.vector.memset(ones32[:], 1.0)
        onesrow = cst.tile([1, 128], f32); nc.vector.memset(onesrow[:], 1.0)
        id32 = cst.tile([128, 128], f32); nc.sync.dma_start(id32[:], idf32[:])
        id16 = cst.tile([128, 128], f16); nc.sync.dma_start(id16[:], idf16[:])
        cosT_s = cst.tile([128, R], f32); nc.sync.dma_start(cosT_s[:], cosT[:])
        sinT_s = cst.tile([128, R], f32); nc.sync.dma_start(sinT_s[:], sinT[:])
        kpe_s = cst.tile([ROPE, T], f16); nc.sync.dma_start(kpe_s[:], kperT[:])
        w_sb = cst.tile([128, 2, IH], f32)
        eps128 = cst.tile([128, 1], f32); nc.vector.memset(eps128[:], EPS)
        eps1 = cst.tile([1, 1], f32); nc.vector.memset(eps1[:], EPS)
        maskT = cst.tile([128, 16, R], f16)
        qhi = cst.tile([128, KT_Q, R], f16)
        kvn = cst.tile([128, KT_KV, T], f16)

        # ================= phase 1: indexer =================
        with tc.tile_pool(name="p1", bufs=1) as p1, \
             tc.tile_pool(name="str1", bufs=2) as st, \
             tc.tile_pool(name="tmp1", bufs=2) as tp, \
             tc.tile_pool(name="dram1", bufs=1, space="DRAM") as drp:
            scores = [p1.tile([128, T], f32, tag=f"sc{i}", name=f"sc{i}") for i in range(2)]
            mask_r = [p1.tile([128, T], f16, tag=f"mk{i}", name=f"mk{i}") for i in range(2)]
            kidxT = p1.tile([128, T], f32)
            khi = p1.tile([128, T], f16)
            klo = p1.tile([128, T], f16)
            qlo = p1.tile([128, KT_Q, R], f16)
            caus_s = [p1.tile([128, T], f16, tag=f"cz{i}", name=f"cz{i}") for i in range(2)]
            for rt in range(2):
                nc.sync.dma_start(caus_s[rt][:], caus_d[rt])
            cosk_s = p1.tile([128, 2, 32], f32)
            nc.sync.dma_start(cosk_s[:], cosK[:].rearrange("(m p) c -> p m c", p=128))
            sink_s = p1.tile([128, 2, 32], f32)
            nc.sync.dma_start(sink_s[:], sinK[:].rearrange("(m p) c -> p m c", p=128))
            gam_s = p1.tile([128, ID], f32); nc.sync.dma_start(gam_s[:], gam_d[:])
            bet_s = p1.tile([128, ID], f32); nc.sync.dma_start(bet_s[:], bet_d[:])
            qcT_s = p1.tile([128, KT_Q, R], f32)
            nc.sync.dma_start(qcT_s[:], qcT[:].rearrange("(k p) r -> p k r", p=128))
            kvcT_s = p1.tile([128, KT_KV, T], f32)
            nc.sync.dma_start(kvcT_s[:], kvcT[:].rearrange("(k p) t -> p k t", p=128))
            rrep = p1.tile([128, R], f32)
            rrep2 = p1.tile([128, T], f32)

            # ---- A: k_idx local fp32 + w_idx; AllGather; transpose ----
            inb = drp.tile([R, ID], f32)
            outb = drp.tile([T, ID], f32)
            with tc.tile_pool(name="ps_kw", bufs=2, space="PSUM") as ps_kw:
                for mt in range(2):
                    ps = ps_kw.tile([128, ID + IH], f32, tag="kw")
                    for kt in range(KT_H):
                        htile = st.tile([128, R], f32, tag="ht")
                        nc.sync.dma_start(htile[:], hT[kt * 128:(kt + 1) * 128, :])
                        wt = st.tile([128, ID + IH], f32, tag="wik")
                        nc.sync.dma_start(wt[:], wikww[kt * 128:(kt + 1) * 128, :])
                        nc.tensor.matmul(ps[:], htile[:, mt * 128:(mt + 1) * 128],
                                         wt[:], start=(kt == 0), stop=(kt == KT_H - 1))
                    nc.vector.tensor_copy(w_sb[:, mt, :], ps[:, ID:])
                    kl = tp.tile([128, ID], f32, tag="kl")
                    mu = tp.tile([128, 1], f32, tag="mu")
                    nc.vector.tensor_reduce(mu[:], ps[:, :ID], AX.X, OP.add)
                    nc.vector.tensor_scalar_mul(mu[:], mu[:], 1.0 / ID)
                    nc.vector.tensor_scalar(kl[:], ps[:, :ID], mu[:], None, OP.subtract)
                    ssq = tp.tile([128, 1], f32, tag="ssq")
                    sq = tp.tile([128, ID], f32, tag="sq")
                    nc.scalar.activation(sq[:], kl[:], AF.Square, accum_out=ssq[:])
                    std = tp.tile([128, 1], f32, tag="std")
                    nc.scalar.activation(std[:], ssq[:], AF.Sqrt, scale=1.0 / ID, bias=eps128[:])
                    rstd = tp.tile([128, 1], f32, tag="rstd")
                    nc.vector.reciprocal(rstd[:], std[:])
                    nc.vector.tensor_scalar(kl[:], kl[:], rstd[:], None, OP.mult)
                    nc.vector.tensor_mul(kl[:], kl[:], gam_s[:])
                    nc.vector.tensor_add(kl[:], kl[:], bet_s[:])
                    t1 = tp.tile([128, 32], f32, tag="kr1")
                    t2 = tp.tile([128, 32], f32, tag="kr2")
                    x1, x2 = kl[:, 0:32], kl[:, 32:64]
                    cc = cosk_s[:, mt, :]
                    ss2 = sink_s[:, mt, :]
                    nc.vector.tensor_mul(t1[:], x1, cc)
                    nc.vector.tensor_mul(t2[:], x2, ss2)
                    nc.vector.tensor_mul(x2, x2, cc)
                    nc.vector.tensor_mul(x1, x1, ss2)
                    nc.vector.tensor_add(x2, x2, x1)
                    nc.vector.tensor_sub(x1, t1[:], t2[:])
                    nc.sync.dma_start(inb[mt * 128:(mt + 1) * 128, :], kl[:])
                nc.gpsimd.collective_compute(
                    "AllGather", OP.bypass, replica_groups=[list(range(NC))],
                    ins=[inb[:]], outs=[outb[:]])
                for jt in range(16):
                    kb = st.tile([128, ID], f32, tag="kb")
                    nc.sync.dma_start(kb[:], outb[jt * 128:(jt + 1) * 128, :])
                    pt = ps_kw.tile([128, 128], f32, tag="ktr")
                    nc.tensor.transpose(pt[:], kb[:], id32[:])
                    nc.scalar.copy(kidxT[:, jt * 128:(jt + 1) * 128], pt[:])
                nc.vector.tensor_copy(khi[:], kidxT[:])
                nc.vector.tensor_sub(klo[:], kidxT[:], khi[:])

            # ---- B: rmsnorm qc (hi/lo) and kv (f16) ----
            with tc.tile_pool(name="ps_sm", bufs=1, space="PSUM") as ps_sm:
                sqp = ps_sm.tile([1, R], f32, tag="sqp")
                for kt in range(KT_Q):
                    sq = tp.tile([128, R], f32, tag="qsq")
                    nc.scalar.activation(sq[:], qcT_s[:, kt, :], AF.Square)
                    nc.tensor.matmul(sqp[:], ones32[:], sq[:],
                                     start=(kt == 0), stop=(kt == KT_Q - 1))
                stdq = tp.tile([1, R], f32, tag="stdq")
                nc.scalar.activation(stdq[:], sqp[:], AF.Sqrt, scale=1.0 / QLR, bias=eps1[:])
                rstdq = tp.tile([1, R], f32, tag="rstdq")
                nc.vector.reciprocal(rstdq[:], stdq[:])
                rp = ps_sm.tile([128, R], f32, tag="rp")
                nc.tensor.matmul(rp[:], onesrow[:], rstdq[:], start=True, stop=True)
                nc.scalar.copy(rrep[:], rp[:])
                for kt in range(KT_Q):
                    qn = tp.tile([128, R], f32, tag="qn")
                    nc.vector.tensor_mul(qn[:], qcT_s[:, kt, :], rrep[:])
                    nc.vector.tensor_copy(qhi[:, kt, :], qn[:])
                    nc.vector.tensor_sub(qlo[:, kt, :], qn[:], qhi[:, kt, :])
                for ncn in range(4):
                    js = ncn * 512
                    svp = ps_sm.tile([1, 512], f32, tag="svp")
                    for kt in range(KT_KV):
                        sq = tp.tile([128, 512], f32, tag="kvsq")
                        nc.scalar.activation(sq[:], kvcT_s[:, kt, js:js + 512], AF.Square)
                        nc.tensor.matmul(svp[:], ones32[:], sq[:],
                                         start=(kt == 0), stop=(kt == KT_KV - 1))
                    stdv = tp.tile([1, 512], f32, tag="stdv")
                    nc.scalar.activation(stdv[:], svp[:], AF.Sqrt,
                                         scale=1.0 / KVLR, bias=eps1[:])
                    rstdv = tp.tile([1, 512], f32, tag="rstdv")
                    nc.vector.reciprocal(rstdv[:], stdv[:])
                    rp2 = ps_sm.tile([128, 512], f32, tag="rp2")
                    nc.tensor.matmul(rp2[:], onesrow[:], rstdv[:], start=True, stop=True)
                    nc.scalar.copy(rrep2[:, js:js + 512], rp2[:])
                for kt in range(KT_KV):
                    nc.vector.tensor_mul(kvn[:, kt, :], kvcT_s[:, kt, :], rrep2[:])

            # ---- C: q_idx (3-term) + rope + split + logits + scores ----
            for i in range(2):
                nc.vector.memset(scores[i][:], 0.0)
            with tc.tile_pool(name="ps_qi", bufs=2, space="PSUM") as ps_qi, \
                 tc.tile_pool(name="ps_lg", bufs=2, space="PSUM") as ps_lg:
                for h in range(IH):
                    ps = ps_qi.tile([128, R], f32, tag="qi")
                    c0 = h * 128
                    wht = st.tile([128, KT_Q, 128], f16, tag="wiqh")
                    nc.sync.dma_start(
                        wht[:], wiq_hi[:, c0:c0 + 128].rearrange(
                            "(k p) c -> p k c", p=128))
                    wlt = st.tile([128, KT_Q, 128], f16, tag="wiql")
                    nc.sync.dma_start(
                        wlt[:], wiq_lo[:, c0:c0 + 128].rearrange(
                            "(k p) c -> p k c", p=128))
                    for kt in range(KT_Q):
                        nc.tensor.matmul(ps[:], wht[:, kt, :], qhi[:, kt, :],
                                         start=(kt == 0), stop=False)
                        nc.tensor.matmul(ps[:], wht[:, kt, :], qlo[:, kt, :],
                                         start=False, stop=False)
                        nc.tensor.matmul(ps[:], wlt[:, kt, :], qhi[:, kt, :],
                                         start=False, stop=(kt == KT_Q - 1))
                    qir = tp.tile([64, R], f32, tag="qir")
                    nc.scalar.copy(qir[:], ps[0:64, :])
                    ta = tp.tile([32, R], f32, tag="rta")
                    tb = tp.tile([32, R], f32, tag="rtb")
                    td = tp.tile([32, R], f32, tag="rtd")
                    te = tp.tile([32, R], f32, tag="rte")
                    x1, x2 = qir[0:32, :], qir[32:64, :]
                    nc.vector.tensor_mul(ta[:], x1, cosT_s[0:32, :])
                    nc.vector.tensor_mul(tb[:], x2, cosT_s[32:64, :])
                    nc.vector.tensor_mul(td[:], x2, sinT_s[32:64, :])
                    nc.vector.tensor_mul(te[:], x1, sinT_s[0:32, :])
                    nc.vector.tensor_sub(x1, ta[:], td[:])
                    nc.vector.tensor_add(ta[:], tb[:], te[:])
                    nc.vector.tensor_copy(x2, ta[:])
                    qih = tp.tile([128, R], f16, tag="qih")
                    qil = tp.tile([128, R], f16, tag="qil")
                    nc.vector.tensor_copy(qih[0:64, :], qir[:])
                    nc.vector.tensor_sub(qil[0:64, :], qir[:], qih[0:64, :])
                    nc.vector.tensor_copy(qih[64:128, :], ps[64:128, :])
                    nc.vector.tensor_sub(qil[64:128, :], ps[64:128, :], qih[64:128, :])
                    for rt in range(2):
                        lq_hi = qih[:, rt * 128:(rt + 1) * 128]
                        lq_lo = qil[:, rt * 128:(rt + 1) * 128]
                        for jc in range(2):
                            pl = ps_lg.tile([128, 1024], f32, tag="lg")
                            for hf in range(2):
                                js = jc * 1024 + hf * 512
                                sl = pl[:, hf * 512:(hf + 1) * 512]
                                nc.tensor.matmul(sl, lq_hi, khi[:, js:js + 512],
                                                 start=True, stop=False)
                                nc.tensor.matmul(sl, lq_hi, klo[:, js:js + 512],
                                                 start=False, stop=False)
                                nc.tensor.matmul(sl, lq_lo, khi[:, js:js + 512],
                                                 start=False, stop=True)
                            so = scores[rt][:, jc * 1024:(jc + 1) * 1024]
                            nc.vector._custom_dve(
                                ACC_W_RELU, out=so, in0=pl[:], in1=so,
                                s0=w_sb[:, rt, h:h + 1])

            # ---- D: threshold search + masks + maskT ----
            with tc.tile_pool(name="ps_tr", bufs=2, space="PSUM") as ps_tr:
                for rt in range(2):
                    sc = scores[rt]
                    rmax = tp.tile([128, 1], f32, tag="rmax")
                    rmin = tp.tile([128, 1], f32, tag="rmin")
                    nc.vector.tensor_reduce(rmax[:], sc[:], AX.X, OP.max)
                    nc.vector.tensor_reduce(rmin[:], sc[:], AX.X, OP.min)
                    nc.vector._custom_dve(MASKSEL, out=sc[:], in0=sc[:],
                                          in1=caus_s[rt][:], s0=NEG)
                    lo = tp.tile([128, 1], f32, tag="lo")
                    hi = tp.tile([128, 1], f32, tag="hi")
                    rng = tp.tile([128, 1], f32, tag="rng")
                    nc.vector.tensor_sub(rng[:], rmax[:], rmin[:])
                    nc.vector.tensor_scalar(lo[:], rng[:], -1e-3, -1e-6, OP.mult, OP.add)
                    nc.vector.tensor_add(lo[:], lo[:], rmin[:])
                    nc.vector.tensor_copy(hi[:], rmax[:])
                    cnt = tp.tile([128, 1], f32, tag="cnt")
                    junk = tp.tile([128, T], f16, tag="junk")
                    cge = tp.tile([128, 1], dt.uint32, tag="cge")
                    clt = tp.tile([128, 1], dt.uint32, tag="clt")
                    mid = tp.tile([128, 1], f32, tag="mid")
                    for _ in range(SEARCH_ITERS):
                        nc.vector.tensor_sub(mid[:], hi[:], lo[:])
                        nc.vector.tensor_scalar(mid[:], mid[:], 0.5, lo[:],
                                                OP.mult, OP.add)
                        nc.vector.tensor_scalar(junk[:], sc[:], mid[:], 0.0,
                                                OP.is_ge, OP.add,
                                                accum_out=cnt[:])
                        nc.vector.tensor_scalar(cge[:], cnt[:], TOPK - 0.5, None,
                                                OP.is_ge)
                        nc.vector.tensor_scalar(clt[:], cnt[:], TOPK - 0.5, None,
                                                OP.is_lt)
                        nc.vector.copy_predicated(lo[:], cge[:], mid[:])
                        nc.vector.copy_predicated(hi[:], clt[:], mid[:])
                    nc.vector.tensor_scalar(mask_r[rt][:], sc[:], lo[:], None, OP.is_ge)
                    nc.vector.tensor_mul(mask_r[rt][:], mask_r[rt][:], caus_s[rt][:])
                for jt in range(16):
                    for rt in range(2):
                        pt = ps_tr.tile([128, 128], f16, tag="mtr")
                        nc.tensor.transpose(pt[:], mask_r[rt][:, jt * 128:(jt + 1) * 128],
                                            id16[:])
                        nc.scalar.copy(maskT[:, jt, rt * 128:(rt + 1) * 128], pt[:])

        # ================= phase 2: attention =================
        with tc.tile_pool(name="p2", bufs=1) as p2, \
             tc.tile_pool(name="str2", bufs=4) as st2, \
             tc.tile_pool(name="tmp2", bufs=2) as tp2, \
             tc.tile_pool(name="dram2", bufs=1, space="DRAM") as drp2:
            qT = p2.tile([128, 24, R], f16)
            qR = p2.tile([64, H, R], f16)
            kvTk = p2.tile([128, 16, T], f16)
            o_sb = p2.tile([128, 16, R], f16)
            vdram = drp2.tile([T, H * VD], f16)

            with tc.tile_pool(name="ps_qp", bufs=3, space="PSUM") as ps_qp:
                for ft in range(24):
                    ps = ps_qp.tile([128, R], f32, tag="qp")
                    wt = st2.tile([128, KT_Q, 128], f16, tag="wqt")
                    nc.sync.dma_start(
                        wt[:], wq[:, ft * 128:(ft + 1) * 128].rearrange(
                            "(k p) c -> p k c", p=128))
                    for kt in range(KT_Q):
                        nc.tensor.matmul(ps[:], wt[:, kt, :], qhi[:, kt, :],
                                         start=(kt == 0), stop=(kt == KT_Q - 1))
                    nc.scalar.copy(qT[:, ft, :], ps[:])
                for hp in range(8):
                    for sub in range(2):
                        b1, b2 = sub * 64, sub * 64 + 32
                        x1 = qT[b1:b1 + 32, 16 + hp, :]
                        x2 = qT[b2:b2 + 32, 16 + hp, :]
                        ta = tp2.tile([32, R], f16, tag="qpa")
                        tb = tp2.tile([32, R], f16, tag="qpb")
                        td = tp2.tile([32, R], f16, tag="qpd")
                        te = tp2.tile([32, R], f16, tag="qpe")
                        nc.vector.tensor_mul(ta[:], x1, cosT_s[b1:b1 + 32, :])
                        nc.vector.tensor_mul(tb[:], x2, cosT_s[b2:b2 + 32, :])
                        nc.vector.tensor_mul(td[:], x2, sinT_s[b2:b2 + 32, :])
                        nc.vector.tensor_mul(te[:], x1, sinT_s[b1:b1 + 32, :])
                        nc.vector.tensor_sub(x1, ta[:], td[:])
                        nc.vector.tensor_add(ta[:], tb[:], te[:])
                        nc.vector.tensor_copy(x2, ta[:])
                        nc.vector.tensor_copy(
                            qR[:, 2 * hp + sub, :],
                            qT[b1:b1 + 64, 16 + hp, :])

            with tc.tile_pool(name="ps_kv", bufs=3, space="PSUM") as ps_kv:
                for ft in range(16):
                    for ncn in range(4):
                        ps = ps_kv.tile([128, 512], f32, tag="kv")
                        for kt in range(KT_KV):
                            wt = st2.tile([128, 128], f16, tag="wkvk")
                            nc.sync.dma_start(wt[:], wkv_k[kt * 128:(kt + 1) * 128,
                                                           ft * 128:(ft + 1) * 128])
                            nc.tensor.matmul(ps[:], wt[:],
                                             kvn[:, kt, ncn * 512:(ncn + 1) * 512],
                                             start=(kt == 0), stop=(kt == KT_KV - 1))
                        nc.scalar.copy(kvTk[:, ft, ncn * 512:(ncn + 1) * 512], ps[:])
                for mt in range(16):
                    for ncn in range(4):
                        ps = ps_kv.tile([128, 512], f32, tag="kv")
                        for kt in range(KT_KV):
                            wt = st2.tile([128, 512], f16, tag="wkvv")
                            nc.sync.dma_start(wt[:], wkv_v[kt * 128:(kt + 1) * 128,
                                                           ncn * 512:(ncn + 1) * 512])
                            nc.tensor.matmul(ps[:], kvn[:, kt, mt * 128:(mt + 1) * 128],
                                             wt[:], start=(kt == 0),
                                             stop=(kt == KT_KV - 1))
                        vt = tp2.tile([128, 512], f16, tag="vev")
                        nc.scalar.copy(vt[:], ps[:])
                        nc.sync.dma_start(vdram[mt * 128:(mt + 1) * 128,
                                                ncn * 512:(ncn + 1) * 512], vt[:])

            with tc.tile_pool(name="ps_att", bufs=1, space="PSUM") as ps_att, \
                 tc.tile_pool(name="ps_o", bufs=1, space="PSUM") as ps_o, \
                 tc.tile_pool(name="ps_s", bufs=1, space="PSUM") as ps_s:
                for g in range(4):
                    o_ps = ps_o.tile([128, 4, 512], f32, tag="o")
                    s_ps = ps_s.tile([128, 4 * R], f32, tag="s")
                    for jt in range(16):
                        att = ps_att.tile([128, 4 * R], f32, tag="att")
                        for i in range(4):
                            h = g * 4 + i
                            sl = att[:, i * R:(i + 1) * R]
                            nc.tensor.matmul(sl, kvTk[:, h, jt * 128:(jt + 1) * 128],
                                             qT[:, h, :], start=True, stop=False)
                            nc.tensor.matmul(
                                sl, kpe_s[:, jt * 128:(jt + 1) * 128],
                                qR[:, h, :], start=False, stop=True)
                        for i in range(4):
                            asl = att[:, i * R:(i + 1) * R]
                            nc.vector._custom_dve(
                                MASKSEL, out=asl, in0=asl,
                                in1=maskT[:, jt, :], s0=-60.0)
                        ee = tp2.tile([128, 4 * R], f16, tag="ee")
                        nc.scalar.activation(ee[:], att[:], AF.Exp)
                        vt = st2.tile([128, H * VD], f16, tag="vt")
                        nc.sync.dma_start(vt[:], vdram[jt * 128:(jt + 1) * 128, :])
                        for i in range(4):
                            h = g * 4 + i
                            nc.tensor.matmul(o_ps[:, i, 0:R],
                                             vt[:, h * 128:(h + 1) * 128],
                                             ee[:, i * R:(i + 1) * R],
                                             start=(jt == 0), stop=(jt == 15))
                        for sc2 in range(2):
                            nc.tensor.matmul(s_ps[:, sc2 * 512:(sc2 + 1) * 512],
                                             ones16[:],
                                             ee[:, sc2 * 512:(sc2 + 1) * 512],
                                             start=(jt == 0), stop=(jt == 15))
                    rec = tp2.tile([128, 4 * R], f32, tag="rec")
                    nc.vector.reciprocal(rec[:], s_ps[:])
                    for i in range(4):
                        h = g * 4 + i
                        nc.vector.tensor_mul(o_sb[:, h, :], o_ps[:, i, 0:R],
                                             rec[:, i * R:(i + 1) * R])

            with tc.tile_pool(name="ps_wo", bufs=2, space="PSUM") as ps_wo:
                for rt in range(2):
                    for ncn in range(14):
                        ps = ps_wo.tile([128, 512], f32, tag="wo")
                        for h in range(16):
                            wt = st2.tile([128, 512], f16, tag="wot")
                            nc.sync.dma_start(wt[:], wo[h * 128:(h + 1) * 128,
                                                        ncn * 512:(ncn + 1) * 512])
                            nc.tensor.matmul(ps[:], o_sb[:, h, rt * 128:(rt + 1) * 128],
                                             wt[:], start=(h == 0), stop=(h == 15))
                        ot = tp2.tile([128, 512], f32, tag="ot")
                        nc.scalar.copy(ot[:], ps[:])
                        nc.sync.dma_start(
                            out_d[rt * 128:(rt + 1) * 128,
                                  ncn * 512:(ncn + 1) * 512], ot[:])
        cst.release()
    nc.finalize()
    return nc


def _noop():
    from contextlib import nullcontext
    return nullcontext()


_NC_CACHE = None


def _host_prep(inputs):
    f32 = np.float32
    pos = np.asarray(inputs["positions"]).astype(f32)
    inv = 1.0 / (ROPE_BASE ** (np.arange(0, ROPE, 2, dtype=f32) / ROPE))
    ang = pos[:, None] * inv
    cos, sin = np.cos(ang).astype(f32), np.sin(ang).astype(f32)
    kpe = np.asarray(inputs["k_pe"]).astype(f32)
    xe, xo = kpe[:, 0::2], kpe[:, 1::2]
    kper = np.concatenate([xe * cos - xo * sin, xo * cos + xe * sin], axis=1)
    kperT = np.ascontiguousarray(kper.T).astype(np.float16)
    qw = np.asarray(inputs["q_a_ln_w"]).astype(f32)
    kvw = np.asarray(inputs["kv_a_ln_w"]).astype(f32)
    wq = np.asarray(inputs["Wq_b"]).astype(f32) * qw[:, None] * SCALE
    wq3 = wq.reshape(QLR, H, NOPE + ROPE)
    nope_part = wq3[:, :, :NOPE].reshape(QLR, H * NOPE)
    rope_part = wq3[:, :, NOPE:]
    ev, od = rope_part[:, :, 0::2], rope_part[:, :, 1::2]
    rope_perm = np.concatenate([ev, od], axis=2).reshape(QLR, H * ROPE)
    wq_perm = np.ascontiguousarray(
        np.concatenate([nope_part, rope_perm], axis=1)).astype(np.float16)
    wkv = np.asarray(inputs["Wkv_b"]).astype(f32) * kvw[:, None]
    wkv3 = wkv.reshape(KVLR, H, NOPE + VD)
    wkv_k = np.ascontiguousarray(
        wkv3[:, :, :NOPE].transpose(0, 1, 2).reshape(KVLR, H * NOPE)).astype(np.float16)
    wkv_v = np.ascontiguousarray(
        wkv3[:, :, NOPE:].reshape(KVLR, H * VD)).astype(np.float16)
    wiq = np.asarray(inputs["Wiq"]).astype(f32) * qw[:, None]
    wiq_hi = wiq.astype(np.float16)
    wiq_lo = (wiq - wiq_hi.astype(f32)).astype(np.float16)
    wikww = np.ascontiguousarray(np.concatenate(
        [np.asarray(inputs["Wik"]).astype(f32),
         np.asarray(inputs["Ww"]).astype(f32)], axis=1))
    wo = np.asarray(inputs["Wo"]).astype(np.float16)
    qcT = np.ascontiguousarray(np.asarray(inputs["q_c"]).astype(f32).T)
    kvcT = np.ascontiguousarray(np.asarray(inputs["kv_c"]).astype(f32).T)
    hTf = np.ascontiguousarray(np.asarray(inputs["hidden"]).astype(f32).T)
    tri01 = np.tril(np.ones((128, 128), np.float16))
    idm = np.eye(128, dtype=f32)
    gam_rep = np.ascontiguousarray(
        np.broadcast_to(np.asarray(inputs["ik_gamma"]).astype(f32), (128, ID)))
    bet_rep = np.ascontiguousarray(
        np.broadcast_to(np.asarray(inputs["ik_beta"]).astype(f32), (128, ID)))
    per_core = []
    for c in range(NC):
        r0 = c * R
        rows = slice(r0, r0 + R)
        causal = np.zeros((2, 128, T), np.float16)
        for rt in range(2):
            gb = r0 + rt * 128
            causal[rt, :, :gb] = 1.0
            causal[rt, :, gb:gb + 128] = tri01
        per_core.append(dict(
            qcT=np.ascontiguousarray(qcT[:, rows]),
            kvcT=kvcT, hT=np.ascontiguousarray(hTf[:, rows]),
            wikww=wikww, wiq_hi=wiq_hi, wiq_lo=wiq_lo, wq=wq_perm,
            wkv_k=wkv_k, wkv_v=wkv_v, wo=wo, kperT=kperT,
            cosT=np.ascontiguousarray(np.tile(cos[rows].T, (4, 1))),
            sinT=np.ascontiguousarray(np.tile(sin[rows].T, (4, 1))),
            cosK=np.ascontiguousarray(cos[rows]),
            sinK=np.ascontiguousarray(sin[rows]),
            gam_rep=gam_rep, bet_rep=bet_rep,
            idf32=idm, idf16=idm.astype(np.float16),
            causal01=causal,
        ))
    return per_core


def kernel(**inputs):
    global _NC_CACHE
    if _NC_CACHE is None:
        _NC_CACHE = build_nc()
    in_maps = _host_prep(inputs)
    res = run_bass_kernel_spmd(_NC_CACHE, in_maps, list(range(NC)))
    out = np.concatenate([res.results[c]["out"] for c in range(NC)], axis=0)
    return np.ascontiguousarray(out.astype(np.float32))



# revision 4
# speedup vs baseline: 2.9518x; 2.9518x over previous
"""DeepseekV3.2 sparse MLA attention — Trainium2 Bass kernel, 8-core SPMD.

Sharding: queries row-sharded (256 rows/core); keys/values replicated.
v2: fp32r indexer (single matmul instead of f16 hi/lo 3-term), phase overlap
(AllGather + DVE-heavy spans hidden under PE projections), batched DMAs,
double-buffered attention PSUM, exp+mask-mul softmax.
Top-k realized as a per-row threshold found by f32 bisection on the DVE.
"""
import numpy as np

import concourse.bass as bass
import concourse.bacc as bacc
import concourse.mybir as mybir
import concourse.tile as tile
import concourse.dve_ops as dve_ops_mod
from concourse.bass_utils import run_bass_kernel_spmd
from concourse.dve_spec import Spec, Src0, Src1, C0, relu, select, lower, spec_leaves
from concourse.dve_ops import DveOp, OPS, get_dve_sub_opcode
from concourse.dve_uop import DveOpSpec

dt = mybir.dt
AF = mybir.ActivationFunctionType
OP = mybir.AluOpType
AX = mybir.AxisListType

T, HID, H, NOPE, ROPE, VD = 2048, 7168, 16, 128, 64, 128
QLR, KVLR, IH, ID, TOPK = 1536, 512, 32, 128, 512
EPS = 1e-6
SCALE = float((NOPE + ROPE) ** -0.5)
ROPE_BASE = 10000.0
NC = 8
R = T // NC            # 256 local rows
NEG = -3.0e38
SEARCH_ITERS = 18
KT_H, KT_Q, KT_KV = HID // 128, QLR // 128, KVLR // 128


def _register(op):
    for o in OPS:
        if o.name == op.name:
            return o
    OPS.append(op)
    dve_ops_mod.CUSTOM_DVE_SPECS[op.name] = op.spec
    dve_ops_mod._SUB_OPCODE_FOR_NAME[op.name] = (
        dve_ops_mod._CUSTOM_DVE_ROW_BASE + len(OPS) - 1)
    for ver in ("v3", "v4"):
        sp = DveOpSpec(name=op.name, opcode=get_dve_sub_opcode(op.name),
                       uops=lower(op.spec, ver=ver),
                       rd1_en=Src1 in spec_leaves(op.spec))
        op.uops_sha[ver] = sp.sha(ver)
    return op


ACC_W_RELU = _register(DveOp(
    "ACC_W_RELU",
    Spec(body=Src1 + C0 * relu(Src0),
         reference=lambda in0, in1, s0, s1, imm2: in1 + s0 * np.maximum(in0, 0)),
    subdim=False, uops_sha={}))

MASKSEL = _register(DveOp(
    "MASKSEL",
    Spec(body=select(Src1, Src0, C0),
         reference=lambda in0, in1, s0, s1, imm2: np.where(in1 != 0, in0, s0)),
    subdim=False, uops_sha={}))


def _emit_os(nc, item, o_ps, s_ps, ones16, R):
    jt, vt, ee = item
    for i in range(4):
        nc.tensor.matmul(o_ps[:, i, 0:R], vt[:, i * 128:(i + 1) * 128],
                         ee[:, i * R:(i + 1) * R],
                         start=(jt == 0), stop=(jt == 15))
    for sc2 in range(2):
        nc.tensor.matmul(s_ps[:, sc2 * 512:(sc2 + 1) * 512], ones16[:],
                         ee[:, sc2 * 512:(sc2 + 1) * 512],
                         start=(jt == 0), stop=(jt == 15))


def build_nc():
    nc = bacc.Bacc(None)
    f32, f16, f32r = dt.float32, dt.float16, dt.float32r

    def din(name, shape, d=f32):
        return nc.dram_tensor(name, list(shape), d, kind="ExternalInput")

    qcT = din("qcT", [QLR, R])
    kvcT = din("kvcT", [KVLR, T])
    hT = din("hT", [HID, R])
    wikww = din("wikww", [HID, ID + IH])
    wiq = din("wiq", [QLR, IH * ID], f32r)
    wq = din("wq", [QLR, H * (NOPE + ROPE)], f16)
    wkv_k = din("wkv_k", [KVLR, H * NOPE], f16)
    wkv_v = din("wkv_v", [KVLR, H * VD], f16)
    wo = din("wo", [H * VD, HID], f16)
    kperT = din("kperT", [ROPE, T], f16)
    cosT = din("cosT", [128, R])
    sinT = din("sinT", [128, R])
    cosK = din("cosK", [R, 32])
    sinK = din("sinK", [R, 32])
    gam_d = din("gam_rep", [128, ID])
    bet_d = din("bet_rep", [128, ID])
    idf32 = din("idf32", [128, 128])
    idf16 = din("idf16", [128, 128], f16)
    caus_d = din("causal01", [2, 128, T], f16)
    out_d = nc.dram_tensor("out", [R, HID], f32, kind="ExternalOutput")

    with tile.TileContext(nc) as tc:
        # ---------- constant pool (whole kernel) ----------
        cst = tc.alloc_tile_pool(name="cst", bufs=1)
        ones16 = cst.tile([128, 128], f16)
        nc.vector.memset(ones16[:], 1.0)
        ones32 = cst.tile([128, 1], f32)
        nc.vector.memset(ones32[:], 1.0)
        onesrow = cst.tile([1, 128], f32)
        nc.vector.memset(onesrow[:], 1.0)
        id32 = cst.tile([128, 128], f32)
        nc.sync.dma_start(id32[:], idf32[:])
        id16 = cst.tile([128, 128], f16)
        nc.sync.dma_start(id16[:], idf16[:])
        cosT_s = cst.tile([128, R], f32)
        nc.sync.dma_start(cosT_s[:], cosT[:])
        sinT_s = cst.tile([128, R], f32)
        nc.sync.dma_start(sinT_s[:], sinT[:])
        kpe_s = cst.tile([ROPE, T], f16)
        nc.sync.dma_start(kpe_s[:], kperT[:])
        eps128 = cst.tile([128, 1], f32)
        nc.vector.memset(eps128[:], EPS)
        eps1 = cst.tile([1, 1], f32)
        nc.vector.memset(eps1[:], EPS)
        w_sb = cst.tile([128, 2, IH], f32)
        neg4 = cst.tile([128, 1], f32)
        nc.vector.memset(neg4[:], -4.0)

        # ---------- persistent tensors ----------
        per = tc.alloc_tile_pool(name="per", bufs=1)
        qTn = per.tile([128, 16, R], f16)      # q nope^T per head-col
        qR = per.tile([64, H, R], f16)         # rope'd q_pe per head
        kvn = per.tile([128, KT_KV, T], f16)   # rmsnorm'd kv_c^T

        idx = tc.alloc_tile_pool(name="idx", bufs=1)   # released at end (LIFO)
        qn = idx.tile([128, KT_Q, R], f32r)    # rmsnorm'd q_c^T (fp32r)
        kidxT = idx.tile([128, T], f32r)       # k_idx^T (fp32r)
        scores = [idx.tile([128, T], f32, name=f"sc{i}") for i in range(2)]

        drp = tc.alloc_tile_pool(name="drp", bufs=1, space="DRAM")
        inb = drp.tile([R, ID], f32)
        outb = drp.tile([T, ID], f32)
        vdram = drp.tile([T, H * VD], f16)

        pd = tc.alloc_tile_pool(name="pd", bufs=1)
        maskT = pd.tile([128, 16, R], f16)
        o_sb = pd.tile([128, 16, R], f16)

        # ================= A: indexer k-side + AllGather =================
        with tc.tile_pool(name="pa", bufs=1) as pa, \
             tc.tile_pool(name="tpa", bufs=2) as tpa, \
             tc.tile_pool(name="ps_a", bufs=1, space="PSUM") as ps_a:
            wik_s = pa.tile([128, KT_H, ID + IH], f32)
            nc.sync.dma_start(
                wik_s[:], wikww[:].rearrange("(k p) c -> p k c", p=128))
            gam_s = pa.tile([128, ID], f32)
            nc.sync.dma_start(gam_s[:], gam_d[:])
            bet_s = pa.tile([128, ID], f32)
            nc.sync.dma_start(bet_s[:], bet_d[:])
            cosk_s = pa.tile([128, 2, 32], f32)
            nc.sync.dma_start(cosk_s[:], cosK[:].rearrange("(m p) c -> p m c", p=128))
            sink_s = pa.tile([128, 2, 32], f32)
            nc.sync.dma_start(sink_s[:], sinK[:].rearrange("(m p) c -> p m c", p=128))
            hTc = pa.tile([128, KT_H // 2, R], f32)
            psm = [ps_a.tile([128, ID + IH], f32, tag=f"kw{mt}", name=f"kw{mt}")
                   for mt in range(2)]
            for ch in range(2):
                k0 = ch * (KT_H // 2)
                nc.sync.dma_start(
                    hTc[:], hT[k0 * 128:(k0 + KT_H // 2) * 128, :].rearrange(
                        "(k p) r -> p k r", p=128))
                for mt in range(2):
                    for kt in range(KT_H // 2):
                        nc.tensor.matmul(
                            psm[mt][:], hTc[:, kt, mt * 128:(mt + 1) * 128],
                            wik_s[:, k0 + kt, :],
                            start=(ch == 0 and kt == 0),
                            stop=(ch == 1 and kt == KT_H // 2 - 1))
            for mt in range(2):
                ps = psm[mt]
                nc.vector.tensor_copy(w_sb[:, mt, :], ps[:, ID:])
                kl = tpa.tile([128, ID], f32, tag="kl")
                mu = tpa.tile([128, 1], f32, tag="mu")
                nc.vector.tensor_reduce(mu[:], ps[:, :ID], AX.X, OP.add)
                nc.vector.tensor_scalar_mul(mu[:], mu[:], 1.0 / ID)
                nc.vector.tensor_scalar(kl[:], ps[:, :ID], mu[:], None, OP.subtract)
                ssq = tpa.tile([128, 1], f32, tag="ssq")
                sq = tpa.tile([128, ID], f32, tag="sq")
                nc.scalar.activation(sq[:], kl[:], AF.Square, accum_out=ssq[:])
                std = tpa.tile([128, 1], f32, tag="std")
                nc.scalar.activation(std[:], ssq[:], AF.Sqrt, scale=1.0 / ID,
                                     bias=eps128[:])
                rstd = tpa.tile([128, 1], f32, tag="rstd")
                nc.vector.reciprocal(rstd[:], std[:])
                nc.vector.tensor_scalar(kl[:], kl[:], rstd[:], None, OP.mult)
                nc.vector.tensor_mul(kl[:], kl[:], gam_s[:])
                nc.vector.tensor_add(kl[:], kl[:], bet_s[:])
                t1 = tpa.tile([128, 32], f32, tag="kr1")
                t2 = tpa.tile([128, 32], f32, tag="kr2")
                x1, x2 = kl[:, 0:32], kl[:, 32:64]
                cc = cosk_s[:, mt, :]
                ss2 = sink_s[:, mt, :]
                nc.vector.tensor_mul(t1[:], x1, cc)
                nc.vector.tensor_mul(t2[:], x2, ss2)
                nc.vector.tensor_mul(x2, x2, cc)
                nc.vector.tensor_mul(x1, x1, ss2)
                nc.vector.tensor_add(x2, x2, x1)
                nc.vector.tensor_sub(x1, t1[:], t2[:])
                nc.sync.dma_start(inb[mt * 128:(mt + 1) * 128, :], kl[:])
            nc.gpsimd.collective_compute(
                "AllGather", OP.bypass, replica_groups=[list(range(NC))],
                ins=[inb[:]], outs=[outb[:]])

        # ================= B: rmsnorm q / kv + qT projection =================
        with tc.tile_pool(name="pb", bufs=1) as pb, \
             tc.tile_pool(name="tpb", bufs=2) as tpb, \
             tc.tile_pool(name="ps_b", bufs=1, space="PSUM") as ps_b:
            qcT_s = pb.tile([128, KT_Q, R], f32)
            nc.sync.dma_start(qcT_s[:], qcT[:].rearrange("(k p) r -> p k r", p=128))
            kvcT_s = pb.tile([128, KT_KV, T], f32)
            nc.sync.dma_start(kvcT_s[:], kvcT[:].rearrange("(k p) t -> p k t", p=128))
            rrep = pb.tile([128, R], f32)
            rrep2 = pb.tile([128, T], f32)
            sqp = ps_b.tile([1, R], f32, tag="sqp")
            for kt in range(KT_Q):
                sq = tpb.tile([128, R], f32, tag="qsq")
                nc.scalar.activation(sq[:], qcT_s[:, kt, :], AF.Square)
                nc.tensor.matmul(sqp[:], ones32[:], sq[:],
                                 start=(kt == 0), stop=(kt == KT_Q - 1))
            stdq = tpb.tile([1, R], f32, tag="stdq")
            nc.scalar.activation(stdq[:], sqp[:], AF.Sqrt, scale=1.0 / QLR,
                                 bias=eps1[:])
            rstdq = tpb.tile([1, R], f32, tag="rstdq")
            nc.vector.reciprocal(rstdq[:], stdq[:])
            rp = ps_b.tile([128, R], f32, tag="rp")
            nc.tensor.matmul(rp[:], onesrow[:], rstdq[:], start=True, stop=True)
            nc.scalar.copy(rrep[:], rp[:])
            for kt in range(KT_Q):
                nc.vector.tensor_mul(qn[:, kt, :], qcT_s[:, kt, :], rrep[:])
            for ncn in range(4):
                js = ncn * 512
                svp = ps_b.tile([1, 512], f32, tag="svp")
                for kt in range(KT_KV):
                    sq = tpb.tile([128, 512], f32, tag="kvsq")
                    nc.scalar.activation(sq[:], kvcT_s[:, kt, js:js + 512], AF.Square)
                    nc.tensor.matmul(svp[:], ones32[:], sq[:],
                                     start=(kt == 0), stop=(kt == KT_KV - 1))
                stdv = tpb.tile([1, 512], f32, tag="stdv")
                nc.scalar.activation(stdv[:], svp[:], AF.Sqrt,
                                     scale=1.0 / KVLR, bias=eps1[:])
                rstdv = tpb.tile([1, 512], f32, tag="rstdv")
                nc.vector.reciprocal(rstdv[:], stdv[:])
                rp2 = ps_b.tile([128, 512], f32, tag="rp2")
                nc.tensor.matmul(rp2[:], onesrow[:], rstdv[:], start=True, stop=True)
                nc.scalar.copy(rrep2[:, js:js + 512], rp2[:])
            for kt in range(KT_KV):
                nc.vector.tensor_mul(kvn[:, kt, :], kvcT_s[:, kt, :], rrep2[:])

            # ---- qT projection ----
            qhi = pb.tile([128, KT_Q, R], f16)
            for kt in range(KT_Q):
                nc.vector.tensor_copy(qhi[:, kt, :], qn[:, kt, :])
            qTr = pb.tile([128, 8, R], f16)
            with tc.tile_pool(name="wqs", bufs=3) as wqs, \
                 tc.tile_pool(name="ps_q", bufs=3, space="PSUM") as ps_q:
                for ft2 in range(12):
                    wt = wqs.tile([128, KT_Q, 256], f16, tag="wq")
                    nc.sync.dma_start(
                        wt[:], wq[:, ft2 * 256:(ft2 + 1) * 256].rearrange(
                            "(k p) c -> p k c", p=128))
                    for fi in range(2):
                        ft = ft2 * 2 + fi
                        ps = ps_q.tile([128, R], f32, tag="qp")
                        for kt in range(KT_Q):
                            nc.tensor.matmul(
                                ps[:], wt[:, kt, fi * 128:(fi + 1) * 128],
                                qhi[:, kt, :],
                                start=(kt == 0), stop=(kt == KT_Q - 1))
                        if ft < 16:
                            nc.scalar.copy(qTn[:, ft, :], ps[:])
                        else:
                            nc.scalar.copy(qTr[:, ft - 16, :], ps[:])
            # rope for q_pe -> qR
            for hp in range(8):
                tt = tpb.tile([128, R], f16, tag="rtt")
                uu = tpb.tile([128, R], f16, tag="ruu")
                nc.vector.tensor_mul(tt[:], qTr[:, hp, :], cosT_s[:])
                for sub in range(2):
                    b1 = sub * 64
                    nc.vector.tensor_mul(uu[b1:b1 + 32, :],
                                         qTr[b1 + 32:b1 + 64, hp, :],
                                         sinT_s[b1 + 32:b1 + 64, :])
                    nc.vector.tensor_mul(uu[b1 + 32:b1 + 64, :],
                                         qTr[b1:b1 + 32, hp, :],
                                         sinT_s[b1:b1 + 32, :])
                for sub in range(2):
                    b1 = sub * 64
                    h = 2 * hp + sub
                    nc.vector.tensor_sub(qR[0:32, h, :], tt[b1:b1 + 32, :],
                                         uu[b1:b1 + 32, :])
                    nc.vector.tensor_add(qR[32:64, h, :], tt[b1 + 32:b1 + 64, :],
                                         uu[b1 + 32:b1 + 64, :])

        # ================= gather -> kidxT transposes =================
        with tc.tile_pool(name="ptr", bufs=1) as ptr, \
             tc.tile_pool(name="ps_tr", bufs=2, space="PSUM") as ps_tr:
            ob = ptr.tile([128, 16, ID], f32)
            nc.sync.dma_start(ob[:], outb[:].rearrange("(j p) d -> p j d", p=128))
            for jt in range(16):
                pt = ps_tr.tile([128, 128], f32, tag="ktr")
                nc.tensor.transpose(pt[:], ob[:, jt, :], id32[:])
                nc.scalar.copy(kidxT[:, jt * 128:(jt + 1) * 128], pt[:])

        # ========== C: indexer scores // P: kv projections (interleaved) ==========
        for i in range(2):
            nc.vector.memset(scores[i][:], 0.0)
        with tc.tile_pool(name="kvp", bufs=1) as kvp:
            kvTk = kvp.tile([128, 16, T], f16)
            with tc.tile_pool(name="kvw", bufs=2) as kvw, \
                 tc.tile_pool(name="wiqs", bufs=2) as wiqs, \
                 tc.tile_pool(name="qiss", bufs=4) as qiss, \
                 tc.tile_pool(name="tpc", bufs=2) as tpc, \
                 tc.tile_pool(name="vts", bufs=2) as vts, \
                 tc.tile_pool(name="ps_qi", bufs=2, space="PSUM") as ps_qi, \
                 tc.tile_pool(name="ps_lg", bufs=2, space="PSUM") as ps_lg, \
                 tc.tile_pool(name="ps_kv", bufs=2, space="PSUM") as ps_kv:
                wkk = wkvv = None
                for h in range(IH):
                    wt = wiqs.tile([128, KT_Q, 128], f32r, tag="wiq")
                    nc.sync.dma_start(
                        wt[:], wiq[:, h * 128:(h + 1) * 128].rearrange(
                            "(k p) c -> p k c", p=128))
                    ps = ps_qi.tile([128, R], f32, tag="qi")
                    for kt in range(KT_Q):
                        nc.tensor.matmul(ps[:], wt[:, kt, :], qn[:, kt, :],
                                         start=(kt == 0), stop=(kt == KT_Q - 1))
                    qis = qiss.tile([128, R], f32r, tag="qis")
                    nc.scalar.copy(qis[:], ps[:])
                    tt = tpc.tile([64, R], f32, tag="ctt")
                    uu = tpc.tile([64, R], f32, tag="cuu")
                    nc.vector.tensor_mul(tt[:], qis[0:64, :], cosT_s[0:64, :])
                    nc.vector.tensor_mul(uu[0:32, :], qis[32:64, :],
                                         sinT_s[32:64, :])
                    nc.vector.tensor_mul(uu[32:64, :], qis[0:32, :],
                                         sinT_s[0:32, :])
                    nc.vector.tensor_sub(qis[0:32, :], tt[0:32, :], uu[0:32, :])
                    nc.vector.tensor_add(qis[32:64, :], tt[32:64, :], uu[32:64, :])
                    for rt in range(2):
                        for jc in range(2):
                            pl = ps_lg.tile([128, 1024], f32, tag="lg")
                            for hf in range(2):
                                js = jc * 1024 + hf * 512
                                nc.tensor.matmul(
                                    pl[:, hf * 512:(hf + 1) * 512],
                                    qis[:, rt * 128:(rt + 1) * 128],
                                    kidxT[:, js:js + 512], start=True, stop=True)
                            so = scores[rt][:, jc * 1024:(jc + 1) * 1024]
                            nc.vector._custom_dve(
                                ACC_W_RELU, out=so, in0=pl[:], in1=so,
                                s0=w_sb[:, rt, h:h + 1])
                    # interleave: 2 kvTk groups + 2 v groups per indexer head
                    if h % 2 == 0:
                        ft = h // 2
                        wkk = kvw.tile([128, KT_KV, 128], f16, tag="wkk")
                        nc.sync.dma_start(
                            wkk[:], wkv_k[:, ft * 128:(ft + 1) * 128].rearrange(
                                "(k p) c -> p k c", p=128))
                    if h % 8 == 0:
                        vcn = h // 8
                        wkvv = kvw.tile([128, KT_KV, 512], f16, tag="wkv")
                        nc.sync.dma_start(
                            wkvv[:], wkv_v[:, vcn * 512:(vcn + 1) * 512].rearrange(
                                "(k p) c -> p k c", p=128))
                    ft, ncn = h // 2, (h % 2) * 2
                    for g2 in range(2):
                        pk = ps_kv.tile([128, 512], f32, tag="kv")
                        for kt in range(KT_KV):
                            nc.tensor.matmul(
                                pk[:], wkk[:, kt, :],
                                kvn[:, kt, (ncn + g2) * 512:(ncn + g2 + 1) * 512],
                                start=(kt == 0), stop=(kt == KT_KV - 1))
                        nc.scalar.copy(
                            kvTk[:, ft, (ncn + g2) * 512:(ncn + g2 + 1) * 512], pk[:])
                    vcn = h // 8
                    for g2 in range(2):
                        mt = (h % 8) * 2 + g2
                        pv = ps_kv.tile([128, 512], f32, tag="kv")
                        for kt in range(KT_KV):
                            nc.tensor.matmul(
                                pv[:], kvn[:, kt, mt * 128:(mt + 1) * 128],
                                wkvv[:, kt, :],
                                start=(kt == 0), stop=(kt == KT_KV - 1))
                        vt = vts.tile([128, 512], f16, tag="vt")
                        nc.scalar.copy(vt[:], pv[:])
                        nc.sync.dma_start(
                            vdram[mt * 128:(mt + 1) * 128,
                                  vcn * 512:(vcn + 1) * 512], vt[:])

            # ================= D: threshold search + maskT =================
            with tc.tile_pool(name="pdc", bufs=1) as pdc, \
                 tc.tile_pool(name="tpd", bufs=1) as tpd:
                caus_s = [pdc.tile([128, T], f16, name=f"cz{i}") for i in range(2)]
                junk = pdc.tile([128, T], f16)
                st = []
                for rt in range(2):
                    nc.sync.dma_start(caus_s[rt][:], caus_d[rt])
                    sc = scores[rt]
                    rmax = tpd.tile([128, 1], f32, tag=f"rmax{rt}", name=f"rmax{rt}")
                    rmin = tpd.tile([128, 1], f32, tag=f"rmin{rt}", name=f"rmin{rt}")
                    nc.vector.tensor_reduce(rmax[:], sc[:], AX.X, OP.max)
                    nc.vector.tensor_reduce(rmin[:], sc[:], AX.X, OP.min)
                    nc.vector._custom_dve(MASKSEL, out=sc[:], in0=sc[:],
                                          in1=caus_s[rt][:], s0=NEG)
                    lo = tpd.tile([128, 1], f32, tag=f"lo{rt}", name=f"lo{rt}")
                    hi = tpd.tile([128, 1], f32, tag=f"hi{rt}", name=f"hi{rt}")
                    rng = tpd.tile([128, 1], f32, tag=f"rng{rt}", name=f"rng{rt}")
                    nc.vector.tensor_sub(rng[:], rmax[:], rmin[:])
                    nc.vector.tensor_scalar(lo[:], rng[:], -1e-3, -1e-6,
                                            OP.mult, OP.add)
                    nc.vector.tensor_add(lo[:], lo[:], rmin[:])
                    nc.vector.tensor_copy(hi[:], rmax[:])
                    cnt = tpd.tile([128, 1], f32, tag=f"cnt{rt}", name=f"cnt{rt}")
                    cge = tpd.tile([128, 1], dt.uint32, tag=f"cge{rt}",
                                   name=f"cge{rt}")
                    clt = tpd.tile([128, 1], dt.uint32, tag=f"clt{rt}",
                                   name=f"clt{rt}")
                    mid = tpd.tile([128, 1], f32, tag=f"mid{rt}", name=f"mid{rt}")
                    st.append((sc, lo, hi, cnt, cge, clt, mid))
                for _ in range(SEARCH_ITERS):
                    for rt in range(2):
                        sc, lo, hi, cnt, cge, clt, mid = st[rt]
                        nc.vector.tensor_sub(mid[:], hi[:], lo[:])
                        nc.vector.tensor_scalar(mid[:], mid[:], 0.5, lo[:],
                                                OP.mult, OP.add)
                        nc.vector.tensor_scalar(junk[:], sc[:], mid[:], 0.0,
                                                OP.is_ge, OP.add, accum_out=cnt[:])
                        nc.vector.tensor_scalar(cge[:], cnt[:], TOPK - 0.5, None,
                                                OP.is_ge)
                        nc.vector.tensor_scalar(clt[:], cnt[:], TOPK - 0.5, None,
                                                OP.is_lt)
                        nc.vector.copy_predicated(lo[:], cge[:], mid[:])
                        nc.vector.copy_predicated(hi[:], clt[:], mid[:])
                for rt in range(2):
                    sc, lo = st[rt][0], st[rt][1]
                    nc.vector.tensor_scalar(junk[:], sc[:], lo[:], None, OP.is_ge)
                    nc.vector.tensor_mul(caus_s[rt][:], caus_s[rt][:], junk[:])
                with tc.tile_pool(name="ps_m", bufs=2, space="PSUM") as ps_m:
                    for jt in range(16):
                        for rt in range(2):
                            pt = ps_m.tile([128, 128], f16, tag="mtr")
                            nc.tensor.transpose(
                                pt[:], caus_s[rt][:, jt * 128:(jt + 1) * 128],
                                id16[:])
                            nc.scalar.copy(maskT[:, jt, rt * 128:(rt + 1) * 128],
                                           pt[:])

            # ================= attention =================
            # NOTE: only ONE matmul accumulation group may be OPEN per PSUM
            # bank at a time (a second group's start=True wipes the open
            # partial).  o regions are bank-aligned; att groups are transient.
            with tc.tile_pool(name="plt", bufs=1) as plt, \
                 tc.tile_pool(name="vls", bufs=4) as vls, \
                 tc.tile_pool(name="ees", bufs=3) as ees, \
                 tc.tile_pool(name="ps_att", bufs=1, space="PSUM") as ps_att, \
                 tc.tile_pool(name="ps_o", bufs=1, space="PSUM") as ps_o, \
                 tc.tile_pool(name="ps_s", bufs=1, space="PSUM") as ps_s:
                rec = plt.tile([128, 4 * R], f32)
                for g in range(4):
                    o_ps = ps_o.tile([128, 4, 512], f32, tag="o")
                    s_ps = ps_s.tile([128, 4 * R], f32, tag="s")
                    ees_q = []
                    for jt in range(16):
                        vt = vls.tile([128, 512], f16, tag="vl")
                        nc.sync.dma_start(
                            vt[:], vdram[jt * 128:(jt + 1) * 128,
                                         g * 512:(g + 1) * 512])
                        ee = ees.tile([128, 4 * R], f16, tag="ee")
                        for half in range(2):
                            atth = ps_att.tile([128, 512], f16 if False else f32,
                                               tag=f"att{half}",
                                               name=f"att{half}")
                            for i2 in range(2):
                                i = half * 2 + i2
                                h = g * 4 + i
                                sl = atth[:, i2 * R:(i2 + 1) * R]
                                nc.tensor.matmul(
                                    sl, kvTk[:, h, jt * 128:(jt + 1) * 128],
                                    qTn[:, h, :], start=True, stop=False)
                                nc.tensor.matmul(
                                    sl, kpe_s[:, jt * 128:(jt + 1) * 128],
                                    qR[:, h, :], start=False, stop=True)
                            nc.scalar.activation(
                                ee[:, half * 512:(half + 1) * 512], atth[:],
                                AF.Exp, bias=neg4[:])
                        for i in range(4):
                            nc.vector.tensor_mul(ee[:, i * R:(i + 1) * R],
                                                 ee[:, i * R:(i + 1) * R],
                                                 maskT[:, jt, :])
                        ees_q.append((jt, vt, ee))
                        if jt >= 1:
                            _emit_os(nc, ees_q.pop(0), o_ps, s_ps, ones16, R)
                    _emit_os(nc, ees_q.pop(0), o_ps, s_ps, ones16, R)
                    nc.vector.reciprocal(rec[:], s_ps[:])
                    for i in range(4):
                        h = g * 4 + i
                        nc.vector.tensor_mul(o_sb[:, h, :], o_ps[:, i, 0:R],
                                             rec[:, i * R:(i + 1) * R])

        # ================= Wo =================
        with tc.tile_pool(name="wos", bufs=3) as wos, \
             tc.tile_pool(name="ots", bufs=4) as ots, \
             tc.tile_pool(name="ps_wo", bufs=2, space="PSUM") as ps_wo:
            ot_cur = [None, None]
            for ncn in range(14):
                wot = wos.tile([128, 16, 512], f16, tag="wo")
                nc.gpsimd.dma_start(
                    wot[:], wo[:, ncn * 512:(ncn + 1) * 512].rearrange(
                        "(k p) c -> p k c", p=128))
                for rt in range(2):
                    ps = ps_wo.tile([128, 512], f32, tag="wo")
                    for h in range(16):
                        nc.tensor.matmul(
                            ps[:], o_sb[:, h, rt * 128:(rt + 1) * 128],
                            wot[:, h, :], start=(h == 0), stop=(h == 15))
                    if ncn % 2 == 0:
                        ot_cur[rt] = ots.tile([128, 1024], f32, tag="ot",
                                              name=f"ot{rt}")
                    nc.scalar.copy(
                        ot_cur[rt][:, (ncn % 2) * 512:(ncn % 2 + 1) * 512], ps[:])
                    if ncn % 2 == 1:
                        nc.sync.dma_start(
                            out_d[rt * 128:(rt + 1) * 128,
                                  (ncn - 1) * 512:(ncn + 1) * 512], ot_cur[rt][:])
        pd.release()
        drp.release()
        idx.release()
        per.release()
        cst.release()
    nc.finalize()
    return nc


_NC_CACHE = None


def _host_prep(inputs):
    f32 = np.float32
    pos = np.asarray(inputs["positions"]).astype(f32)
    inv = 1.0 / (ROPE_BASE ** (np.arange(0, ROPE, 2, dtype=f32) / ROPE))
    ang = pos[:, None] * inv
    cos, sin = np.cos(ang).astype(f32), np.sin(ang).astype(f32)
    kpe = np.asarray(inputs["k_pe"]).astype(f32)
    xe, xo = kpe[:, 0::2], kpe[:, 1::2]
    kper = np.concatenate([xe * cos - xo * sin, xo * cos + xe * sin], axis=1)
    kperT = np.ascontiguousarray(kper.T).astype(np.float16)
    qw = np.asarray(inputs["q_a_ln_w"]).astype(f32)
    kvw = np.asarray(inputs["kv_a_ln_w"]).astype(f32)
    wq = np.asarray(inputs["Wq_b"]).astype(f32) * qw[:, None] * SCALE
    wq3 = wq.reshape(QLR, H, NOPE + ROPE)
    nope_part = wq3[:, :, :NOPE].reshape(QLR, H * NOPE)
    rope_part = wq3[:, :, NOPE:]
    ev, od = rope_part[:, :, 0::2], rope_part[:, :, 1::2]
    rope_perm = np.concatenate([ev, od], axis=2).reshape(QLR, H * ROPE)
    wq_perm = np.ascontiguousarray(
        np.concatenate([nope_part, rope_perm], axis=1)).astype(np.float16)
    wkv = np.asarray(inputs["Wkv_b"]).astype(f32) * kvw[:, None]
    wkv3 = wkv.reshape(KVLR, H, NOPE + VD)
    wkv_k = np.ascontiguousarray(
        wkv3[:, :, :NOPE].reshape(KVLR, H * NOPE)).astype(np.float16)
    wkv_v = np.ascontiguousarray(
        wkv3[:, :, NOPE:].reshape(KVLR, H * VD)).astype(np.float16)
    wiq = np.ascontiguousarray(
        np.asarray(inputs["Wiq"]).astype(f32) * qw[:, None])
    wikww = np.ascontiguousarray(np.concatenate(
        [np.asarray(inputs["Wik"]).astype(f32),
         np.asarray(inputs["Ww"]).astype(f32)], axis=1))
    wo = np.asarray(inputs["Wo"]).astype(np.float16)
    qcT = np.ascontiguousarray(np.asarray(inputs["q_c"]).astype(f32).T)
    kvcT = np.ascontiguousarray(np.asarray(inputs["kv_c"]).astype(f32).T)
    hTf = np.ascontiguousarray(np.asarray(inputs["hidden"]).astype(f32).T)
    tri01 = np.tril(np.ones((128, 128), np.float16))
    idm = np.eye(128, dtype=f32)
    gam_rep = np.ascontiguousarray(
        np.broadcast_to(np.asarray(inputs["ik_gamma"]).astype(f32), (128, ID)))
    bet_rep = np.ascontiguousarray(
        np.broadcast_to(np.asarray(inputs["ik_beta"]).astype(f32), (128, ID)))
    per_core = []
    for c in range(NC):
        r0 = c * R
        rows = slice(r0, r0 + R)
        causal = np.zeros((2, 128, T), np.float16)
        for rt in range(2):
            gb = r0 + rt * 128
            causal[rt, :, :gb] = 1.0
            causal[rt, :, gb:gb + 128] = tri01
        per_core.append(dict(
            qcT=np.ascontiguousarray(qcT[:, rows]),
            kvcT=kvcT, hT=np.ascontiguousarray(hTf[:, rows]),
            wikww=wikww, wiq=wiq, wq=wq_perm,
            wkv_k=wkv_k, wkv_v=wkv_v, wo=wo, kperT=kperT,
            cosT=np.ascontiguousarray(np.tile(cos[rows].T, (4, 1))),
            sinT=np.ascontiguousarray(np.tile(sin[rows].T, (4, 1))),
            cosK=np.ascontiguousarray(cos[rows]),
            sinK=np.ascontiguousarray(sin[rows]),
            gam_rep=gam_rep, bet_rep=bet_rep,
            idf32=idm, idf16=idm.astype(np.float16),
            causal01=causal,
        ))
    return per_core


def kernel(**inputs):
    global _NC_CACHE
    if _NC_CACHE is None:
        _NC_CACHE = build_nc()
    in_maps = _host_prep(inputs)
    res = run_bass_kernel_spmd(_NC_CACHE, in_maps, list(range(NC)))
    out = np.concatenate([res.results[c]["out"] for c in range(NC)], axis=0)
    return np.ascontiguousarray(out.astype(np.float32))
